# revision 13
# baseline (speedup 1.0000x reference)
"""Bipartite graph convolution (GCMC-style) Trainium2 kernel, 8-core SPMD.

Math (reference): per-rating masks M_r = (adj == r), r=1..5,
  out_u = relu(d_u * sum_r (M_r @ v_feat) @ W_u[r]),  d_u = 1/deg_u
  out_v = relu(d_v * sum_r (M_r.T @ u_feat) @ W_v[r]), d_v = 1/deg_v

Device formulation (per core, u-rows sharded 1024/core):
  Fold weights on host: P_r = v_feat @ W_u[r], Q_r = u_feat_shard @ W_v[r].
  Basis trick: since adj = sum_r r*M_r,
    sum_r M_r X_r = adj @ X_1 + sum_{r=2..5} M_r @ (X_r - r*X_1)
  so only 4 on-chip is_equal mask passes per orientation (adj tile itself is
  the 5th stationary operand). A 65th moving column carries per-basis
  constants (+1 for adj, -(r-1) for M_r) so PSUM col 64 accumulates the
  degree (edge count) for free.
  PE: stationary = [128u x 128v] fp16 mask/adj tile, moving = [128, 65]
  fp16 features+deg -> PSUM f32 [128, 65].
  Phase A (out_u): lhsT = adjT tiles (streamed), 8 persistent PSUM banks
  accumulate the whole u-shard; finish deg/relu on-chip.
  Phase B (out_v): lhsT = adj tiles (resident), 8 PSUM banks per v-group of
  8, partial [8192, 65] DMA'd out; host all-reduces over cores + finishes.
"""

import numpy as np
import sys

sys.path.insert(0, "/opt/trn_rl_repo")

N_U, N_V = 8192, 8192
F = 64
R = 5
N_CORES = 8
U_SH = N_U // N_CORES          # 1024 rows per core
UC = U_SH // 128               # 8 u-chunks per core
VC = N_V // 128                # 64 v-chunks
VG = 8                         # v-groups of 8 chunks (phase B)
J = F + 1                      # 64 features + degree column

_CACHE = {}


def _build():
    import concourse.bass as bass
    import concourse.bacc as bacc
    import concourse.mybir as mybir
    import concourse.tile as tile

    dt = mybir.dt
    eq = mybir.AluOpType.is_equal
    mx = mybir.AluOpType.max
    mult = mybir.AluOpType.mult

    nc = bacc.Bacc("TRN2", target_bir_lowering=False, debug=False,
                   num_devices=N_CORES)

    adj_h = nc.dram_tensor("adj_h", [U_SH, N_V], dt.float16,
                           kind="ExternalInput").ap()
    adjt_h = nc.dram_tensor("adjt_h", [N_V, U_SH], dt.float16,
                            kind="ExternalInput").ap()
    q_mov = nc.dram_tensor("q_mov", [128, R * UC * J], dt.float16,
                           kind="ExternalInput").ap()
    p_mov = nc.dram_tensor("p_mov", [128, R * VC * J], dt.float16,
                           kind="ExternalInput").ap()
    out_u = nc.dram_tensor("out_u_part", [U_SH, F], dt.float32,
                           kind="ExternalOutput").ap()
    out_v = nc.dram_tensor("out_v_part", [N_V, J], dt.float32,
                           kind="ExternalOutput").ap()

    with tile.TileContext(nc) as tc:
        with tc.tile_pool(name="consts", bufs=1) as cons, \
             tc.tile_pool(name="adjres", bufs=1) as adjres, \
             tc.tile_pool(name="adjts", bufs=3) as adjts, \
             tc.tile_pool(name="masks", bufs=2) as masks, \
             tc.tile_pool(name="fin", bufs=4) as fin:

            # SBUF budget/partition (192KB cap): adj resident 8x16K=128K
            # (phase B, DMA'd during A as space allows) + adjt stream 3x2K
            # + masks 2x8K + p stream (per-vc [128, R*J]) + q_t 5.1K.
            q_t = cons.tile([128, R * UC * J], dt.float16, tag="q")
            nc.sync.dma_start(q_t[:], q_mov[:])

            # resident adj tiles for phase B (DMA early, consumed later)
            adj_t = []
            for uc in range(UC):
                t = adjres.tile([128, N_V], dt.float16, tag=f"adj{uc}")
                nc.sync.dma_start(t[:], adj_h[uc * 128:(uc + 1) * 128, :])
                adj_t.append(t)

            # ---------------- Phase A: out_u ----------------
            pspA = tc.tile_pool(name="psumA", bufs=1, space="PSUM")
            psp = pspA.__enter__()
            ps_u = [psp.tile([128, J], dt.float32, tag=f"psu{uc}",
                             name=f"psu{uc}") for uc in range(UC)]
            for vc in range(VC):
                at = adjts.tile([128, U_SH], dt.float16, tag="adjt")
                nc.sync.dma_start(at[:], adjt_h[vc * 128:(vc + 1) * 128, :])
                pt = masks.tile([128, R * J], dt.float16, tag="pstream")
                nc.sync.dma_start(pt[:], p_mov[:, vc * R * J:(vc + 1) * R * J])
                mt = masks.tile([128, 4 * U_SH], dt.float16, tag="mask")
                for k in range(4):
                    nc.vector.tensor_scalar(
                        mt[:, k * U_SH:(k + 1) * U_SH], at[:],
                        float(k + 2), None, op0=eq)
                for uc in range(UC):
                    for b in range(R):
                        if b == 0:
                            lhsT = at[:, uc * 128:(uc + 1) * 128]
                        else:
                            lhsT = mt[:, (b - 1) * U_SH + uc * 128:
                                      (b - 1) * U_SH + (uc + 1) * 128]
                        nc.tensor.matmul(
                            ps_u[uc][:], lhsT,
                            pt[:, b * J:(b + 1) * J],
                            start=(vc == 0 and b == 0),
                            stop=(vc == VC - 1 and b == R - 1))
            # finish out_u: d_u = 1/max(deg,0.5); relu(d_u * x)
            for uc in range(UC):
                dtl = fin.tile([128, 1], dt.float32, tag="deg")
                nc.vector.tensor_scalar(dtl[:], ps_u[uc][:, F:F + 1], 0.5,
                                        None, op0=mx)
                rtl = fin.tile([128, 1], dt.float32, tag="rec")
                nc.vector.reciprocal(rtl[:], dtl[:])
                otl = fin.tile([128, F], dt.float32, tag="outu")
                nc.vector.tensor_scalar(otl[:], ps_u[uc][:, 0:F], rtl[:, 0:1],
                                        0.0, op0=mult, op1=mx)
                nc.sync.dma_start(out_u[uc * 128:(uc + 1) * 128, :], otl[:])

            pspA.__exit__(None, None, None)

            # ---------------- Phase B: out_v partial ----------------
            pspB = tc.tile_pool(name="psumB", bufs=1, space="PSUM")
            psp = pspB.__enter__()
            for vg in range(VG):
                ps_v = [psp.tile([128, J], dt.float32, tag=f"psv{i}",
                                 name=f"psv{vg}_{i}") for i in range(8)]
                for uc in range(UC):
                    mt = masks.tile([128, 4 * 1024], dt.float16, tag="mask")
                    src = adj_t[uc][:, vg * 1024:(vg + 1) * 1024]
                    for k in range(4):
                        nc.vector.tensor_scalar(
                            mt[:, k * 1024:(k + 1) * 1024], src,
                            float(k + 2), None, op0=eq)
                    for i in range(8):
                        for b in range(R):
                            if b == 0:
                                lhsT = adj_t[uc][:, (vg * 8 + i) * 128:
                                                 (vg * 8 + i + 1) * 128]
                            else:
                                lhsT = mt[:, (b - 1) * 1024 + i * 128:
                                          (b - 1) * 1024 + (i + 1) * 128]
                            nc.tensor.matmul(
                                ps_v[i][:], lhsT,
                                q_t[:, (b * UC + uc) * J:(b * UC + uc + 1) * J],
                                start=(uc == 0 and b == 0),
                                stop=(uc == UC - 1 and b == R - 1))
                for i in range(8):
                    vc = vg * 8 + i
                    ev = fin.tile([128, J], dt.float32, tag="evac",
                                  name=f"ev{vg}_{i}")
                    nc.scalar.copy(ev[:], ps_v[i][:])
                    nc.sync.dma_start(out_v[vc * 128:(vc + 1) * 128, :],
                                      ev[:])
            pspB.__exit__(None, None, None)

    nc.compile()
    return nc


def _host_prep(adj, u_feature, v_feature, weight_u, weight_v):
    adj = np.asarray(adj)
    u_feature = np.asarray(u_feature, dtype=np.float32)
    v_feature = np.asarray(v_feature, dtype=np.float32)
    weight_u = np.asarray(weight_u, dtype=np.float32)
    weight_v = np.asarray(weight_v, dtype=np.float32)

    adj16 = adj.astype(np.float16)

    # P_r = v_feat @ W_u[r]  (phase A moving), Q_r = u_shard @ W_v[r] (phase B)
    P = np.einsum("vf,rfo->rvo", v_feature, weight_u)      # [R, N_V, F]
    # basis transform: X^_1 = X_1 ; X^_r = X_r - r*X_1 (r=2..5)
    Pb = np.empty((R, N_V, J), np.float32)
    Pb[0, :, :F] = P[0]
    Pb[0, :, F] = 1.0
    for r in range(2, R + 1):
        Pb[r - 1, :, :F] = P[r - 1] - r * P[0]
        Pb[r - 1, :, F] = -(r - 1)
    # p_mov[p, (vc*R+b)*J + j] = Pb[b, vc*128+p, j]  (vc-major for streaming)
    p_mov = np.ascontiguousarray(
        Pb.reshape(R, VC, 128, J).transpose(2, 1, 0, 3).reshape(128, R * VC * J)
    ).astype(np.float16)

    in_maps = []
    for c in range(N_CORES):
        sl = slice(c * U_SH, (c + 1) * U_SH)
        Q = np.einsum("uf,rfo->ruo", u_feature[sl], weight_v)  # [R, U_SH, F]
        Qb = np.empty((R, U_SH, J), np.float32)
        Qb[0, :, :F] = Q[0]
        Qb[0, :, F] = 1.0
        for r in range(2, R + 1):
            Qb[r - 1, :, :F] = Q[r - 1] - r * Q[0]
            Qb[r - 1, :, F] = -(r - 1)
        q_mov = np.ascontiguousarray(
            Qb.reshape(R, UC, 128, J).transpose(2, 0, 1, 3)
            .reshape(128, R * UC * J)).astype(np.float16)
        a = adj16[sl]
        in_maps.append({
            "adj_h": np.ascontiguousarray(a),
            "adjt_h": np.ascontiguousarray(a.T),
            "q_mov": q_mov,
            "p_mov": p_mov,
        })
    return in_maps


def kernel(adj, u_feature, v_feature, weight_u, weight_v, _trace=False):
    from concourse import bass_utils

    if "nc" not in _CACHE:
        _CACHE["nc"] = _build()
    nc = _CACHE["nc"]

    in_maps = _host_prep(adj, u_feature, v_feature, weight_u, weight_v)
    res = bass_utils.run_bass_kernel_spmd(
        nc, in_maps, core_ids=list(range(N_CORES)), trace=_trace)
    _CACHE["last_result"] = res

    out_u = np.concatenate([res.results[c]["out_u_part"]
                            for c in range(N_CORES)], axis=0)
    acc = np.zeros((N_V, J), np.float64)
    for c in range(N_CORES):
        acc += res.results[c]["out_v_part"]
    acc = acc.astype(np.float32)
    deg_v = acc[:, F]
    d_v = np.where(deg_v > 0, 1.0 / np.maximum(deg_v, 0.5), 0.0)
    out_v = np.maximum(acc[:, :F] * d_v[:, None], 0.0).astype(np.float32)
    return out_u, out_v
